# revision 1
# baseline (speedup 1.0000x reference)
"""Gemma sliding-window GQA attention block on 8 TRN2 NeuronCores.

Sharding: core = (batch b in {0,1}) x (kv head k in {0..3}). Each core
computes its kv head + the 2 grouped q heads for one batch and produces a
partial output projection [2048, 2304]; the host sums the 4 kv-head partials
per batch.

Layouts on device (per core):
  QT, KT : [head_dim(=2x128 chunks) partitions, t free]  (from projection)
  V      : [t partitions, head_dim+ones free]            (ones col -> rowsum)
  ST     : [j partitions, i free] logits tiles; tanh softcap + exp on ACT
  O      : [i partitions, head_dim+rowsum free] PSUM accumulation over j
  OT     : [h partitions, t free] via PE transpose after 1/rowsum normalize
  out    : [t partitions, d free] = OT.T @ Wo summed over 4 h-chunks

All matmuls run as float32r (fp22 mantissa truncation, full PE rate).
RoPE's Gemma interleave permutation is skipped (identical permutation on q
and k cancels in q.k); 1/sqrt(head_dim) is folded into Wq on the host.
"""

import sys

if "/opt/trn_rl_repo" not in sys.path:
    sys.path.insert(0, "/opt/trn_rl_repo")

import numpy as np

import concourse.bass as bass
import concourse.mybir as mybir
import concourse.tile as tile_mod
from concourse.bass_utils import run_bass_kernel_spmd
from concourse.tile import ScopedClock, TileContext

F32 = mybir.dt.float32
F32R = mybir.dt.float32r

T = 2048
D = 2304
HD = 256          # head dim
P = 128
DC = D // P       # 18 contraction chunks
NT = T // P       # 16 t-tiles
NIB = T // 256    # 8 i-blocks of 256 queries
WTILES = 8        # sliding window of 1024 = 8 tiles of 128
SOFT_CAP = 50.0
D_BLOCKS = [(0, 512), (512, 512), (1024, 512), (1536, 512), (2048, 256)]


def _patched_drain_and_barrier(self, tick_clock, wait_clock):
    # walrus CTRL codegen rejects >1 sem wait on one Drain; spread the
    # tail-drain waits across one drain instruction per wait.
    nc = self.nc
    drain_inst = nc.sync.drain()
    wait_clock.add_sem_waits(
        drain_inst.ins, ScopedClock({None: tick_clock.global_clock})
    )
    si = drain_inst.ins.sync_info
    if si is not None and si.on_wait and len(si.on_wait) > 1:
        extra = list(si.on_wait[1:])
        del si.on_wait[1:]
        for w in extra:
            nxt = nc.sync.drain()
            nsi = nxt.ins.sync_info
            if nsi is None:
                nxt.ins.sync_info = mybir.SyncInfo(on_wait=[w], on_update=[])
            else:
                nsi.on_wait.append(w)

    nc.all_engine_barrier()
    assert self.sems is not None
    popped = nc._tile_sem_poison_stack.pop()
    assert popped is self._sem_poison
    nc.clear_and_free_semaphores(list(self.sems.allocated().values()))
    nc.all_engine_barrier()


tile_mod.TileContext._drain_and_barrier = _patched_drain_and_barrier


def r(ap):
    return ap.bitcast(F32R)


def build_program(repeat=1):
    nc = bass.Bass()
    xt = nc.declare_dram_parameter("xt", [D, T], F32, isOutput=False)
    wq = nc.declare_dram_parameter("wq", [2, D, HD], F32, isOutput=False)
    wk = nc.declare_dram_parameter("wk", [D, HD], F32, isOutput=False)
    wv = nc.declare_dram_parameter("wv", [D, HD], F32, isOutput=False)
    wo = nc.declare_dram_parameter("wo", [2 * HD, D], F32, isOutput=False)
    cos = nc.declare_dram_parameter("cos", [P, T], F32, isOutput=False)
    sin = nc.declare_dram_parameter("sin", [P, T], F32, isOutput=False)
    tri = nc.declare_dram_parameter("tri", [P, P], F32, isOutput=False)
    wedge = nc.declare_dram_parameter("wedge", [P, P], F32, isOutput=False)
    ident = nc.declare_dram_parameter("ident", [P, P], F32, isOutput=False)
    ones = nc.declare_dram_parameter("ones", [P, 2], F32, isOutput=False)
    out = nc.declare_dram_parameter("out", [T, D], F32, isOutput=True)

    hw_dma = nc.sync if repeat == 1 else nc.gpsimd
    Tanh = mybir.ActivationFunctionType.Tanh
    Exp = mybir.ActivationFunctionType.Exp

    with TileContext(nc) as tc:
      with tc.tile_pool(name="persist", bufs=1) as persist:
        # ---- persistent SBUF tensors ----
        tri_t = persist.tile([P, P], F32, tag="tri", name="tri")
        nc.sync.dma_start(out=tri_t[:, :], in_=tri[:, :])
        wedge_t = persist.tile([P, P], F32, tag="wedge", name="wedge")
        nc.sync.dma_start(out=wedge_t[:, :], in_=wedge[:, :])
        ident_t = persist.tile([P, P], F32, tag="ident", name="ident")
        nc.sync.dma_start(out=ident_t[:, :], in_=ident[:, :])

        # head-interleaved Q: col(ib, hq, il) = ib*512 + hq*256 + il
        qt_c = [persist.tile([P, 2 * T], F32, tag=f"qtc{c}", name=f"qtc{c}")
                for c in (0, 1)]
        kt_t = [persist.tile([P, T], F32, tag=f"kt{c}", name=f"kt{c}")
                for c in (0, 1)]
        v_t = []
        for tt in range(NT):
            tv = persist.tile([P, HD + 2], F32, tag=f"v{tt}", name=f"v{tt}")
            nc.sync.dma_start(out=r(tv[:, HD:HD + 2]), in_=r(ones[:, :]))
            v_t.append(tv)

        # ---- phase A: projections (QT/KT in [h,t], V in [t,h]) + RoPE ----
        for rep in range(repeat):
          with (
              tc.tile_pool(name="wkv", bufs=1) as wkv_pool,
              tc.tile_pool(name="xts", bufs=2) as xt_pool,
              tc.tile_pool(name="wqs", bufs=2) as wq_pool,
              tc.tile_pool(name="qkp", bufs=1, space="PSUM") as qk_psum,
              tc.tile_pool(name="vp", bufs=1, space="PSUM") as v_psum,
              tc.tile_pool(name="rope", bufs=2) as rope_pool,
          ):
              wk_t = []
              wv_t = []
              for dc in range(DC):
                  tk = wkv_pool.tile([P, HD], F32, tag=f"wk{dc}", name=f"wk{dc}")
                  hw_dma.dma_start(out=r(tk[:, :]),
                                    in_=r(wk[dc * P:(dc + 1) * P, :]))
                  wk_t.append(tk)
                  tv = wkv_pool.tile([P, HD], F32, tag=f"wv{dc}", name=f"wv{dc}")
                  hw_dma.dma_start(out=r(tv[:, :]),
                                    in_=r(wv[dc * P:(dc + 1) * P, :]))
                  wv_t.append(tv)
              for ts in range(4):
                  qk_ps = [qk_psum.tile([P, 512], F32, tag=f"qk{m}",
                                        name=f"qk{m}") for m in range(6)]
                  v_ps = [v_psum.tile([P, 512], F32, tag=f"vps{m}",
                                      name=f"vps{m}") for m in range(2)]
                  for g6 in range(3):          # 6-d-chunk macro groups
                      wq_g = wq_pool.tile([P, 3072], F32, tag="wq", name="wq_g")
                      for hq in range(2):
                          nc.gpsimd.dma_start(
                              out=r(wq_g[:, hq * 1536:(hq + 1) * 1536]
                                    ).rearrange("p (k c) -> p k c", k=6),
                              in_=r(wq[hq, 6 * g6 * P:(6 * g6 + 6) * P, :]
                                    ).rearrange("(k p) c -> p k c", p=P),
                          )
                      for half in range(2):    # 3-d-chunk xt groups
                          g = 2 * g6 + half
                          xt_g = xt_pool.tile([P, 1536], F32, tag="xt",
                                              name="xt_g")
                          nc.gpsimd.dma_start(
                              out=r(xt_g[:, :]).rearrange(
                                  "p (g t) -> p g t", g=3),
                              in_=r(xt[3 * g * P:(3 * g + 3) * P,
                                       ts * 512:(ts + 1) * 512]).rearrange(
                                  "(g p) t -> p g t", p=P),
                          )
                          for k3 in range(3):
                              dc = 3 * g + k3
                              k6 = dc - 6 * g6
                              st = dc == 0
                              sp = dc == DC - 1
                              xt_sl = xt_g[:, k3 * 512:(k3 + 1) * 512]
                              for m in range(4):  # QT m-tiles (hq, c)
                                  mh, mc = divmod(m, 2)
                                  nc.tensor.matmul(
                                      qk_ps[m][:, :],
                                      r(wq_g[:, mh * 1536 + k6 * HD + mc * P:
                                             mh * 1536 + k6 * HD +
                                             (mc + 1) * P]),
                                      r(xt_sl),
                                      start=st, stop=sp,
                                  )
                              for c in range(2):   # KT
                                  nc.tensor.matmul(
                                      qk_ps[4 + c][:, :],
                                      r(wk_t[dc][:, c * P:(c + 1) * P]),
                                      r(xt_sl),
                                      start=st, stop=sp,
                                  )
                              for tt2 in range(4):  # V [t,h]
                                  nc.tensor.matmul(
                                      v_ps[tt2 // 2][:, (tt2 % 2) * HD:
                                                     (tt2 % 2 + 1) * HD],
                                      r(xt_g[:, k3 * 512 + tt2 * P:
                                             k3 * 512 + (tt2 + 1) * P]),
                                      r(wv_t[dc][:, :]),
                                      start=st and tt2 % 2 == 0,
                                      stop=sp and tt2 % 2 == 1,
                                  )
                  sl = slice(ts * 512, (ts + 1) * 512)
                  for m in range(4):
                      mh, mc = divmod(m, 2)
                      dst = r(qt_c[mc][:, :]).rearrange(
                          "p (b h i) -> p b h i", h=2, i=256)[
                          :, 2 * ts:2 * ts + 2, mh, :]
                      nc.scalar.copy(
                          dst,
                          qk_ps[m][:, :].rearrange("p (b i) -> p b i", i=256))
                  for c in range(2):
                      nc.scalar.copy(r(kt_t[c][:, sl]), qk_ps[4 + c][:, :])
                  for tt2 in range(4):
                      nc.vector.tensor_copy(
                          r(v_t[ts * 4 + tt2][:, 0:HD]),
                          v_ps[tt2 // 2][:, (tt2 % 2) * HD:(tt2 % 2 + 1) * HD])

                  # RoPE on this t-slice (in place, rotate-half)
                  cos_t = rope_pool.tile([P, 512], F32, tag="cos", name="cos_t")
                  nc.gpsimd.dma_start(out=cos_t[:, :], in_=cos[:, sl])
                  sin_t = rope_pool.tile([P, 512], F32, tag="sin", name="sin_t")
                  nc.gpsimd.dma_start(out=sin_t[:, :], in_=sin[:, sl])
                  cos3 = cos_t[:, :].rearrange("p (b i) -> p b i", i=256)
                  sin3 = sin_t[:, :].rearrange("p (b i) -> p b i", i=256)
                  qv = [r(qt_c[c][:, :]).rearrange(
                      "p (b h i) -> p b h i", h=2, i=256) for c in (0, 1)]
                  pairs = [(qv[0][:, 2 * ts:2 * ts + 2, hq, :],
                            qv[1][:, 2 * ts:2 * ts + 2, hq, :],
                            cos3, sin3) for hq in (0, 1)]
                  pairs.append((r(kt_t[0][:, sl]), r(kt_t[1][:, sl]),
                                cos_t[:, :], sin_t[:, :]))
                  for a, b, cc, ss in pairs:
                      s1 = rope_pool.tile([P, 512], F32, tag="s1", name="s1")
                      s13 = (s1[:, :].rearrange("p (b i) -> p b i", i=256)
                             if len(a.shape) == 3 else s1[:, :])
                      s2 = rope_pool.tile([P, 512], F32, tag="s2", name="s2")
                      s23 = (s2[:, :].rearrange("p (b i) -> p b i", i=256)
                             if len(a.shape) == 3 else s2[:, :])
                      nc.vector.tensor_mul(s13, a, ss)
                      nc.vector.tensor_mul(s23, b, ss)
                      nc.vector.tensor_mul(a, a, cc)
                      nc.vector.tensor_sub(a, a, s23)
                      nc.vector.tensor_mul(b, b, cc)
                      nc.vector.tensor_add(b, b, s13)

          wo_t = []
          for ch in range(4):
              wt = persist.tile([P, D], F32, tag=f"wo{ch}", name=f"wo{ch}")
              hw_dma.dma_start(out=r(wt[:, :]),
                                in_=r(wo[ch * P:(ch + 1) * P, :]))
              wo_t.append(wt)

          # ---- phase B: banded attention + output projection, per i-block ----
          with (
              tc.tile_pool(name="stp", bufs=2, space="PSUM") as st_psum,
              tc.tile_pool(name="op", bufs=1, space="PSUM") as o_psum,
              tc.tile_pool(name="outp", bufs=2, space="PSUM") as out_psum,
              tc.tile_pool(name="pb", bufs=3) as p_pool,
              tc.tile_pool(name="otr", bufs=2) as ot_pool,
              tc.tile_pool(name="small", bufs=4) as small_pool,
              tc.tile_pool(name="outs", bufs=2) as out_pool,
          ):
              pending = []

              def outproj_group(ot_prev, tt, d_idx, ob):
                  d0, dn = D_BLOCKS[d_idx]
                  ps = out_psum.tile([P, dn], F32, tag="ops", name="ops")
                  for ch in range(4):
                      nc.tensor.matmul(
                          ps[:, :],
                          r(ot_prev[ch][:, (tt % 2) * P:(tt % 2 + 1) * P]),
                          r(wo_t[ch][:, d0:d0 + dn]),
                          start=(ch == 0), stop=(ch == 3),
                      )
                  nc.vector.tensor_copy(ob[:, d0:d0 + dn], ps[:, :])
                  if d_idx == len(D_BLOCKS) - 1:
                      nc.gpsimd.dma_start(
                          out=out[tt * P:(tt + 1) * P, :], in_=ob[:, :])

              def queue_outproj(ot_prev, ib_prev):
                  for tt in (2 * ib_prev, 2 * ib_prev + 1):
                      ob = out_pool.tile([P, D], F32, tag="ob", name="ob")
                      for d_idx in range(len(D_BLOCKS)):
                          pending.append(
                              (outproj_group, ot_prev, tt, d_idx, ob))

              def pop_pending(k=1):
                  for _ in range(k):
                      if pending:
                          fn, *args = pending.pop(0)
                          fn(*args)

              for ib in range(NIB):
                  jlo = max(0, 2 * ib - WTILES)
                  jhi = 2 * ib + 1
                  qsl = slice(ib * 512, (ib + 1) * 512)
                  ot_ib = [ot_pool.tile([P, 256], F32, tag=f"ot{ch}",
                                        name=f"ot{ch}") for ch in range(4)]
                  o_ps = [[o_psum.tile([P, HD + 2], F32, tag=f"o{hq}{k}",
                                       name=f"o{hq}{k}") for k in range(2)]
                          for hq in range(2)]
                  for jt in range(jlo, jhi + 1):
                      jsl = slice(jt * P, (jt + 1) * P)
                      st_t = st_psum.tile([P, 512], F32, tag="st",
                                          name="st_t")
                      for c in range(2):
                          nc.tensor.matmul(
                              st_t[:, :],
                              r(kt_t[c][:, jsl]),
                              r(qt_c[c][:, qsl]),
                              start=(c == 0), stop=(c == 1),
                          )
                      nc.scalar.activation(
                          st_t[:, :], st_t[:, :], Tanh, scale=1.0 / SOFT_CAP)
                      pb_t = p_pool.tile([P, 512], F32, tag="pb", name="pb_t")
                      nc.scalar.activation(
                          r(pb_t[:, :]), st_t[:, :], Exp, scale=SOFT_CAP)
                      for s in range(2):
                          dd = 2 * ib + s - jt
                          if dd == 0:
                              m_t = tri_t
                          elif dd == WTILES:
                              m_t = wedge_t
                          else:
                              m_t = None
                          for hq in range(2):
                              csl = slice(hq * 256 + s * P,
                                          hq * 256 + (s + 1) * P)
                              if m_t is not None:
                                  nc.vector.tensor_mul(
                                      r(pb_t[:, csl]), pb_t[:, csl],
                                      m_t[:, :])
                              elif dd < 0 or dd > WTILES:
                                  nc.vector.tensor_scalar_mul(
                                      r(pb_t[:, csl]), pb_t[:, csl], 0.0)
                      for hq in range(2):
                          for k in range(2):
                              nc.tensor.matmul(
                                  o_ps[hq][k][:, :],
                                  r(pb_t[:, hq * 256 + k * P:
                                         hq * 256 + (k + 1) * P]),
                                  r(v_t[jt][:, :]),
                                  start=(jt == jlo), stop=(jt == jhi),
                              )
                      pop_pending()
                  for hq in range(2):
                      for k in range(2):
                          rec = small_pool.tile([P, 1], F32, tag="rec",
                                                name="rec")
                          nc.vector.reciprocal(rec[:, :],
                                               o_ps[hq][k][:, HD:HD + 1])
                          osb = small_pool.tile([P, HD], F32, tag="osb",
                                                name="osb")
                          nc.vector.tensor_scalar_mul(
                              osb[:, :], o_ps[hq][k][:, 0:HD], rec[:, :])
                          for c in range(2):
                              tp = out_psum.tile([P, P], F32, tag="ops",
                                                 name="tp")
                              nc.tensor.transpose(
                                  tp[:, :], osb[:, c * P:(c + 1) * P],
                                  ident_t[:, :])
                              nc.vector.tensor_copy(
                                  r(ot_ib[hq * 2 + c][:, k * P:(k + 1) * P]),
                                  tp[:, :])
                  # queue this i-block's output projection; flush rest
                  pop_pending(len(pending))
                  queue_outproj(ot_ib, ib)
              pop_pending(len(pending))

    _split_excess_waits(nc)
    return nc


def _split_excess_waits(nc, max_waits=1):
    """Walrus codegen allows few sem-wait slots per engine instruction (1 for
    CTRL / S3_LW structs). Move excess waits onto same-engine NOPs inserted
    right before the offending instruction."""
    all_blocks = [bb for f in nc.m.functions for bb in f.blocks]
    for bb in all_blocks:
        insts = bb.instructions
        i = 0
        while i < len(insts):
            inst = insts[i]
            si = inst.sync_info
            if si is not None and si.on_wait and len(si.on_wait) > max_waits:
                tname = type(inst).__name__
                eng = getattr(inst, "engine", None)
                if eng is None or (
                        "DMA" in tname and eng == mybir.EngineType.SP):
                    i += 1
                    continue
                waits = list(si.on_wait)
                keep = waits[-max_waits:]
                extra = waits[:-max_waits]
                del si.on_wait[:]
                si.on_wait.extend(keep)
                pos = i
                for j in range(0, len(extra), max_waits):
                    chunk = extra[j:j + max_waits]
                    nop_b = nc.engines[eng].nop(nofuse=True)
                    nop_inst = nop_b.ins
                    for bb2 in all_blocks:
                        lst = bb2.instructions
                        if lst and lst[-1] is nop_inst:
                            lst.pop()
                            break
                    nop_inst.sync_info = mybir.SyncInfo(
                        on_wait=list(chunk), on_update=[])
                    insts.insert(pos, nop_inst)
                    pos += 1
                    i += 1
            i += 1


_CACHE = {}


def _get_program():
    if "nc" not in _CACHE:
        _CACHE["nc"] = build_program()
    return _CACHE["nc"]


def _host_inputs(x, Wq, Wk, Wv, Wo):
    inv_freq = (1.0 / (10000.0 ** (np.arange(0, HD, 2, dtype=np.float32)
                                   / np.float32(HD)))).astype(np.float32)
    pos = np.arange(T, dtype=np.float32)
    freq = (inv_freq[:, None] * pos[None, :]).astype(np.float32)  # [128, T]
    cos = np.cos(freq).astype(np.float32)
    sin = np.sin(freq).astype(np.float32)

    jj = np.arange(P)[:, None]
    ii = np.arange(P)[None, :]
    tri = (ii >= jj).astype(np.float32)
    wedge = (ii < jj).astype(np.float32)
    ident = np.eye(P, dtype=np.float32)

    scale = np.float32(1.0 / np.sqrt(HD))
    in_maps = []
    for b in range(2):
        xt = np.ascontiguousarray(x[b].T)
        for k in range(4):
            in_maps.append({
                "xt": xt,
                "wq": np.ascontiguousarray(Wq[2 * k:2 * k + 2]) * scale,
                "wk": np.ascontiguousarray(Wk[k]),
                "wv": np.ascontiguousarray(Wv[k]),
                "wo": np.ascontiguousarray(
                    Wo[2 * k:2 * k + 2].reshape(2 * HD, D)),
                "cos": cos,
                "sin": sin,
                "tri": tri,
                "wedge": wedge,
                "ident": ident,
                "ones": np.ones((P, 2), np.float32),
            })
    return in_maps


def _run(x, Wq, Wk, Wv, Wo, trace=False):
    nc = _get_program()
    in_maps = _host_inputs(x, Wq, Wk, Wv, Wo)
    res = run_bass_kernel_spmd(nc, in_maps, list(range(8)), trace=trace)
    outs = [res.results[i]["out"] for i in range(8)]
    full = np.stack([
        outs[0] + outs[1] + outs[2] + outs[3],
        outs[4] + outs[5] + outs[6] + outs[7],
    ], axis=0)
    return full, res


def kernel(x, attention_mask, Wq, Wk, Wv, Wo):
    x = np.asarray(x, dtype=np.float32)
    full, _ = _run(x, np.asarray(Wq, dtype=np.float32),
                   np.asarray(Wk, dtype=np.float32),
                   np.asarray(Wv, dtype=np.float32),
                   np.asarray(Wo, dtype=np.float32))
    return full

